# revision 35
# baseline (speedup 1.0000x reference)
"""Multi-head attention block (B=8, S=1024, D=1024, H=16) on 8 TRN2 NeuronCores.

Data-parallel over batch: core i computes batch element i end-to-end.
Per-core dataflow (bf16 compute, f32 PSUM accumulation; x/W pre-cast to
bf16 on the host):
  xT = transpose(x)                      (PE transposes)
  qkT[n,s] = W_qkv[:, :2048]^T @ x^T     (q rows pre-scaled by hd^-0.5)
  v[s,n]   = x @ W_qkv[:, 2048:]         (stored head-interleaved with a
                                          ones column per head -> "va", M=65)
  per head: scoresT[kj,qi] = kT^T q      (2 heads row-packed, K=64 each)
            expT = exp(scoresT)          (ScalarE, PSUM -> SBUF bf16)
            outT[c,qi], Z[qi] = va^T @ expT   (ones column accumulates Z)
            outT /= Z                    (approx reciprocal + partition bcast)
  out = outT^T @ W_proj + b_proj
QKV matmuls are interleaved pair-by-pair with attention so the PE never
idles while ScalarE works through the exps (keeps the HAM clock warm).
"""

import sys

if "/opt/trn_rl_repo" not in sys.path:
    sys.path.insert(0, "/opt/trn_rl_repo")

import ml_dtypes
import numpy as np

P = 128
S = 1024
D = 1024
H = 16
HD = 64
N_CORES = 8
SCALE = HD ** -0.5
ST = S // P   # 8 s-tiles
DT = D // P   # 8 d-tiles (contraction tiles)

_CACHE = {}


def _build():
    if "nc" in _CACHE:
        return _CACHE["nc"]

    from contextlib import ExitStack

    import concourse.bass as bass  # noqa: F401
    import concourse.mybir as mybir
    import concourse.tile as tile
    from concourse import bacc
    F32 = mybir.dt.float32
    BF = mybir.dt.bfloat16
    AluOp = mybir.AluOpType
    Act = mybir.ActivationFunctionType

    nc = bacc.Bacc(
        "TRN2", target_bir_lowering=False, debug=False, num_devices=N_CORES
    )

    x_d = nc.dram_tensor("x", [D, S], BF, kind="ExternalInput")  # x^T
    wqkv_d = nc.dram_tensor("W_qkv", [D, 3 * D], BF, kind="ExternalInput")
    bqkv_d = nc.dram_tensor("b_qkv", [3 * D], F32, kind="ExternalInput")
    wproj_d = nc.dram_tensor("W_proj", [D, D], BF, kind="ExternalInput")
    bproj_d = nc.dram_tensor("b_proj", [D], F32, kind="ExternalInput")
    out_d = nc.dram_tensor("out", [S, D], F32, kind="ExternalOutput")

    with tile.TileContext(nc) as tc, ExitStack() as ctx:
        const = ctx.enter_context(tc.tile_pool(name="const", bufs=1))
        persist = ctx.enter_context(tc.tile_pool(name="persist", bufs=1))
        # PSUM: "big" [128,1024]f32 tiles (2 banks) x3 + "pso" [65,512] x2
        psum = ctx.enter_context(tc.tile_pool(name="psum", bufs=3, space="PSUM"))
        psmall = ctx.enter_context(tc.tile_pool(name="psmall", bufs=2, space="PSUM"))
        small = ctx.enter_context(tc.tile_pool(name="small", bufs=2))

        # ---- constants ----
        zbias = const.tile([P, 1], F32)  # zero bias for activation(Exp)
        nc.gpsimd.memset(zbias[:], 0.0)

        # b_qkv q,k part: host passes it permuted to [p, nt] layout -> one DMA
        bqcol = const.tile([P, 16], F32)

        def load_biases():
            nc.sync.dma_start(bqcol[:], bqkv_d[: 2 * D].rearrange("(p t) -> p t", t=16))

        # ---- persistent tensors ----
        # qkT: only 2 pairs live at a time -> 4 rotating slots
        qk_pool = ctx.enter_context(tc.tile_pool(name="qk", bufs=4))
        va = [persist.tile([P, H * (HD + 1)], BF, name=f"va{s}") for s in range(ST)]
        outT = [persist.tile([P, S], BF, name=f"outT{t}") for t in range(DT)]
        xT = [persist.tile([P, S], BF, name=f"xT{t}") for t in range(DT)]
        WqkE = [persist.tile([P, 4 * P], BF, name=f"WqkE{t}") for t in range(DT)]
        Wqk = [persist.tile([P, 2 * D - 4 * P], BF, name=f"Wqk{t}", tag=f"wsh{t}") for t in range(DT)]

        for s8 in range(ST):
            nc.gpsimd.memset(va[s8][:], 1.0)  # ones columns survive the v copies

        # ---- DMA loads (bf16, pre-cast + pre-transposed x on host) ----
        # interleaved so qkv(0)'s per-k-tile deps complete progressively
        for dt2 in range(DT):
            nc.sync.dma_start(xT[dt2][:], x_d[dt2 * P : (dt2 + 1) * P, :])
            nc.sync.dma_start(
                WqkE[dt2][:, 0 : 2 * P], wqkv_d[dt2 * P : (dt2 + 1) * P, 0 : 2 * P]
            )
            nc.sync.dma_start(
                WqkE[dt2][:, 2 * P : 4 * P],
                wqkv_d[dt2 * P : (dt2 + 1) * P, D : D + 2 * P],
            )
        load_biases()


        qkT = {}

        def qkv_pair(hp):
            """qkT tiles for pair hp: q (scaled) and k, 2 rotating slots."""
            qt = qk_pool.tile([P, S], BF, name=f"q{hp}", tag="qk")
            kt = qk_pool.tile([P, S], BF, name=f"k{hp}", tag="qk")
            qkT[hp] = (qt, kt)
            for nt, dst in ((hp, qt), (8 + hp, kt)):
                ps = psum.tile([P, S], F32, name="ps_qk", tag="big")
                for sh in range(2):
                    for dt2 in range(DT):
                        if hp < 2:
                            w_ap = WqkE[dt2][
                                :, ((nt >= 8) * 2 + hp) * P : ((nt >= 8) * 2 + hp + 1) * P
                            ]
                        else:
                            col = (nt - 2) if nt < 8 else (nt - 4)
                            w_ap = Wqk[dt2][:, col * P : (col + 1) * P]
                        nc.tensor.matmul(
                            ps[:, sh * 512 : (sh + 1) * 512],
                            w_ap,
                            xT[dt2][:, sh * 512 : (sh + 1) * 512],
                            start=(dt2 == 0),
                            stop=(dt2 == DT - 1),
                        )
                if nt < 8:  # q: (psum + b) * scale
                    nc.vector.tensor_scalar(
                        dst[:], ps[:], bqcol[:, nt : nt + 1], SCALE,
                        AluOp.add, AluOp.mult,
                    )
                else:  # k: psum + b
                    nc.vector.tensor_scalar_add(dst[:], ps[:], bqcol[:, nt : nt + 1])

        def v_phase(Wv):
            # v = x @ Wv + bv, head-interleaved into va
            for s8 in range(ST):
                ps = psum.tile([P, S], F32, name="ps_v", tag="big")
                for sh in range(2):
                    for dt2 in range(DT):
                        nc.tensor.matmul(
                            ps[:, sh * 512 : (sh + 1) * 512],
                            xT[dt2][:, s8 * P : (s8 + 1) * P],
                            Wv[dt2][:, sh * 512 : (sh + 1) * 512],
                            start=(dt2 == 0),
                            stop=(dt2 == DT - 1),
                        )
                nc.vector.tensor_copy(
                    va[s8][:].rearrange("p (h c) -> p h c", c=HD + 1)[:, :, 0:HD],
                    ps[:].rearrange("p (h c) -> p h c", c=HD),
                )

        exp_tiles = {}

        def scores_pair(hp, exp_pool):
            """scoresT + exp for heads (2hp, 2hp+1); fills exp_tiles[hp]."""
            expA = exp_pool.tile([P, ST * S], BF, name="expA", tag="expA")
            expB = exp_pool.tile([P, ST * S], BF, name="expB", tag="expB")
            exp_tiles[hp] = (expA, expB)
            qtile, ktile = qkT[hp]
            for jt in range(ST):
                psA = psum.tile([P, S], F32, name="psA", tag="big")
                psB = psum.tile([P, S], F32, name="psB", tag="big")
                for sh in range(2):
                    nc.tensor.matmul(
                        psA[:, sh * 512 : (sh + 1) * 512],
                        ktile[0:64, jt * P : (jt + 1) * P],
                        qtile[0:64, sh * 512 : (sh + 1) * 512],
                        tile_position=(0, 0),
                    )
                    nc.tensor.matmul(
                        psB[:, sh * 512 : (sh + 1) * 512],
                        ktile[64:128, jt * P : (jt + 1) * P],
                        qtile[64:128, sh * 512 : (sh + 1) * 512],
                        tile_position=(64, 0),
                    )
                nc.scalar.activation(
                    expA[:, jt * S : (jt + 1) * S], psA[:], Act.Exp, bias=zbias[:]
                )
                nc.scalar.activation(
                    expB[:, jt * S : (jt + 1) * S], psB[:], Act.Exp, bias=zbias[:]
                )

        def attnv_pair(hp):
            expA, expB = exp_tiles.pop(hp)
            for qh in range(2):
                for (ex, head) in ((expA, 2 * hp), (expB, 2 * hp + 1)):
                    pso = psmall.tile([HD + 1, 512], F32, name="pso", tag="pso")
                    for jt in range(ST):
                        nc.tensor.matmul(
                            pso[:],
                            va[jt][:, head * 65 : head * 65 + 65],
                            ex[:, jt * S + qh * 512 : jt * S + qh * 512 + 512],
                            start=(jt == 0),
                            stop=(jt == ST - 1),
                        )
                    po = (head % 2) * 64
                    reg = outT[hp][po : po + 64, qh * 512 : (qh + 1) * 512]
                    # copy unnormalized rows + Z out fast to release PSUM
                    nc.vector.tensor_copy(reg, pso[0:64, :])
                    zs = small.tile([1, 512], F32, name="zs", tag="zs")
                    nc.vector.tensor_copy(zs[:], pso[64:65, :])
                    rz = small.tile([1, 512], F32, name="rz", tag="rz")
                    nc.vector.reciprocal_approx_fast(out=rz[:], in_=zs[:])
                    bz = small.tile([P, 512], F32, name="bz", tag="bz")
                    nc.gpsimd.partition_broadcast(bz[:], rz[:])
                    nc.vector.tensor_mul(reg, reg, bz[po : po + 64, :])

        # ---- output projection (defs; emitted at schedule time) ----
        ob_pool = ctx.enter_context(tc.tile_pool(name="obp", bufs=2))
        # Wp reuses the Wqk slots (dead after qkv pair 7) via shared tags
        Wp = [
            persist.tile([P, D], BF, name=f"Wp{t}", tag=f"wsh{t}") for t in range(DT)
        ]

        def load_wp():
            for dt2 in range(DT):
                nc.sync.dma_start(Wp[dt2][:], wproj_d[dt2 * P : (dt2 + 1) * P, :])

        proj_ps = {}

        def proj_partial():
            # pairs 0-6 are done; accumulate their proj contribution for
            # st 0,1 while the PE would otherwise wait on exp(7)
            for st in range(2):
                ps = psum.tile([P, S], F32, name="ps_p", tag="big")
                proj_ps[st] = ps
                for sh in range(2):
                    for kt in range(DT - 1):
                        nc.tensor.matmul(
                            ps[:, sh * 512 : (sh + 1) * 512],
                            outT[kt][:, st * P : (st + 1) * P],
                            Wp[kt][:, sh * 512 : (sh + 1) * 512],
                            start=(kt == 0),
                            stop=False,
                        )

        def proj_emit(st, ps, kt0):
            for sh in range(2):
                for kt in range(kt0, DT):
                    nc.tensor.matmul(
                        ps[:, sh * 512 : (sh + 1) * 512],
                        outT[kt][:, st * P : (st + 1) * P],
                        Wp[kt][:, sh * 512 : (sh + 1) * 512],
                        start=(kt == 0),
                        stop=(kt == DT - 1),
                    )
            ob = ob_pool.tile([P, S], F32, name="ob", tag="ob")
            for sh in range(2):
                nc.vector.tensor_copy(
                    ob[:, sh * 512 : (sh + 1) * 512], ps[:, sh * 512 : (sh + 1) * 512]
                )
                nc.sync.dma_start(
                    out_d[st * P : (st + 1) * P, sh * 512 : (sh + 1) * 512],
                    ob[:, sh * 512 : (sh + 1) * 512],
                )

        # ---- software-pipelined schedule ----
        with tc.tile_pool(name="xv", bufs=1) as xv_pool, \
             tc.tile_pool(name="exp", bufs=2) as exp_pool:
            Wv = [xv_pool.tile([P, D], BF, name=f"Wv{t}") for t in range(DT)]
            for dt2 in range(DT):
                nc.sync.dma_start(
                    Wv[dt2][:], wqkv_d[dt2 * P : (dt2 + 1) * P, 2 * D :]
                )
            # bulk Wqk (pairs 2-7) after Wv: needed only ~70us in
            for dt2 in range(DT):
                nc.sync.dma_start(
                    Wqk[dt2][:, 0 : D - 2 * P],
                    wqkv_d[dt2 * P : (dt2 + 1) * P, 2 * P : D],
                )
                nc.sync.dma_start(
                    Wqk[dt2][:, D - 2 * P :],
                    wqkv_d[dt2 * P : (dt2 + 1) * P, D + 2 * P : 2 * D],
                )
            qkv_pair(0)
            qkv_pair(1)
            scores_pair(0, exp_pool)
            v_phase(Wv)
            for hp in range(1, 8):
                scores_pair(hp, exp_pool)
                if hp + 1 < 8:
                    qkv_pair(hp + 1)
                attnv_pair(hp - 1)
            load_wp()
            proj_partial()
            attnv_pair(7)
            for st in range(ST):
                if st in proj_ps:
                    proj_emit(st, proj_ps[st], DT - 1)
                else:
                    ps = psum.tile([P, S], F32, name="ps_p", tag="big")
                    proj_emit(st, ps, 0)

    nc.compile()
    _CACHE["nc"] = nc
    return nc


def kernel(x, W_qkv, b_qkv, W_proj, b_proj, _trace=False):
    nc = _build()
    from concourse.bass_utils import run_bass_kernel_spmd

    bf = ml_dtypes.bfloat16
    wq = np.ascontiguousarray(W_qkv, dtype=np.float32).astype(bf)
    wp = np.ascontiguousarray(W_proj, dtype=np.float32).astype(bf)
    bq0 = np.asarray(b_qkv, dtype=np.float32)
    bq = np.concatenate(
        [np.ascontiguousarray(bq0[:2048].reshape(16, 128).T).ravel(), bq0[2048:]]
    ).astype(np.float32)
    bp = np.ascontiguousarray(b_proj, dtype=np.float32)
    in_maps = []
    for i in range(N_CORES):
        in_maps.append(
            {
                "x": np.ascontiguousarray(np.asarray(x[i], dtype=np.float32).T).astype(bf),
                "W_qkv": wq,
                "b_qkv": bq,
                "W_proj": wp,
                "b_proj": bp,
            }
        )
    res = run_bass_kernel_spmd(
        nc, in_maps, core_ids=list(range(N_CORES)), trace=_trace
    )
    out = np.stack([res.results[i]["out"] for i in range(N_CORES)], axis=0).astype(
        np.float32
    )
    # v-bias and proj-bias applied exactly on the host:
    # out = (attn + 1*bv) @ Wp + bp  ==  attn @ Wp  +  (bv @ Wp + bp)
    corr = np.asarray(b_qkv, np.float32)[2 * D :] @ np.asarray(W_proj, np.float32)
    corr = corr + np.asarray(b_proj, np.float32)
    if np.any(corr):
        out += corr[None, None, :]
    if _trace:
        _CACHE["last_results"] = res
    return out


# revision 36
# speedup vs baseline: 1.0361x; 1.0361x over previous
"""Multi-head attention block (B=8, S=1024, D=1024, H=16) on 8 TRN2 NeuronCores.

Data-parallel over batch: core i computes batch element i end-to-end.
Per-core dataflow (bf16 compute, f32 PSUM accumulation; x/W pre-cast to
bf16 on the host):
  xT = transpose(x)                      (PE transposes)
  qkT[n,s] = W_qkv[:, :2048]^T @ x^T     (q rows pre-scaled by hd^-0.5)
  v[s,n]   = x @ W_qkv[:, 2048:]         (stored head-interleaved with a
                                          ones column per head -> "va", M=65)
  per head: scoresT[kj,qi] = kT^T q      (2 heads row-packed, K=64 each)
            expT = exp(scoresT)          (ScalarE, PSUM -> SBUF bf16)
            outT[c,qi], Z[qi] = va^T @ expT   (ones column accumulates Z)
            outT /= Z                    (approx reciprocal + partition bcast)
  out = outT^T @ W_proj + b_proj
QKV matmuls are interleaved pair-by-pair with attention so the PE never
idles while ScalarE works through the exps (keeps the HAM clock warm).
"""

import sys

if "/opt/trn_rl_repo" not in sys.path:
    sys.path.insert(0, "/opt/trn_rl_repo")

import ml_dtypes
import numpy as np

P = 128
S = 1024
D = 1024
H = 16
HD = 64
N_CORES = 8
SCALE = HD ** -0.5
ST = S // P   # 8 s-tiles
DT = D // P   # 8 d-tiles (contraction tiles)

_CACHE = {}


def _build():
    if "nc" in _CACHE:
        return _CACHE["nc"]

    from contextlib import ExitStack

    import concourse.bass as bass  # noqa: F401
    import concourse.mybir as mybir
    import concourse.tile as tile
    from concourse import bacc
    F32 = mybir.dt.float32
    BF = mybir.dt.bfloat16
    AluOp = mybir.AluOpType
    Act = mybir.ActivationFunctionType

    nc = bacc.Bacc(
        "TRN2", target_bir_lowering=False, debug=False, num_devices=N_CORES
    )

    x_d = nc.dram_tensor("x", [D, S], BF, kind="ExternalInput")  # x^T
    wqkv_d = nc.dram_tensor("W_qkv", [D, 3 * D], BF, kind="ExternalInput")
    bqkv_d = nc.dram_tensor("b_qkv", [3 * D], F32, kind="ExternalInput")
    wproj_d = nc.dram_tensor("W_proj", [D, D], BF, kind="ExternalInput")
    bproj_d = nc.dram_tensor("b_proj", [D], F32, kind="ExternalInput")
    out_d = nc.dram_tensor("out", [S, D], F32, kind="ExternalOutput")

    with tile.TileContext(nc) as tc, ExitStack() as ctx:
        const = ctx.enter_context(tc.tile_pool(name="const", bufs=1))
        persist = ctx.enter_context(tc.tile_pool(name="persist", bufs=1))
        # PSUM: "big" [128,1024]f32 tiles (2 banks) x3 + "pso" [65,512] x2
        psum = ctx.enter_context(tc.tile_pool(name="psum", bufs=3, space="PSUM"))
        psmall = ctx.enter_context(tc.tile_pool(name="psmall", bufs=2, space="PSUM"))
        small = ctx.enter_context(tc.tile_pool(name="small", bufs=2))

        # ---- constants ----
        zbias = const.tile([P, 1], F32)  # zero bias for activation(Exp)
        nc.gpsimd.memset(zbias[:], 0.0)

        # b_qkv q,k part: host passes it permuted to [p, nt] layout -> one DMA
        bqcol = const.tile([P, 16], F32)

        def load_biases():
            nc.sync.dma_start(bqcol[:], bqkv_d[: 2 * D].rearrange("(p t) -> p t", t=16))

        # ---- persistent tensors ----
        # qkT: only 2 pairs live at a time -> 4 rotating slots
        qk_pool = ctx.enter_context(tc.tile_pool(name="qk", bufs=4))
        va = [persist.tile([P, H * (HD + 1)], BF, name=f"va{s}") for s in range(ST)]
        outT = [persist.tile([P, S], BF, name=f"outT{t}") for t in range(DT)]
        xT = [persist.tile([P, S], BF, name=f"xT{t}") for t in range(DT)]
        WqkE = [persist.tile([P, 4 * P], BF, name=f"WqkE{t}") for t in range(DT)]
        Wqk = [persist.tile([P, 2 * D - 4 * P], BF, name=f"Wqk{t}", tag=f"wsh{t}") for t in range(DT)]

        for s8 in range(ST):
            nc.gpsimd.memset(va[s8][:], 1.0)  # ones columns survive the v copies

        # ---- DMA loads (bf16, pre-cast + pre-transposed x on host) ----
        # interleaved so qkv(0)'s per-k-tile deps complete progressively
        for dt2 in range(DT):
            nc.sync.dma_start(xT[dt2][:], x_d[dt2 * P : (dt2 + 1) * P, :])
            nc.sync.dma_start(
                WqkE[dt2][:, 0 : 2 * P], wqkv_d[dt2 * P : (dt2 + 1) * P, 0 : 2 * P]
            )
            nc.sync.dma_start(
                WqkE[dt2][:, 2 * P : 4 * P],
                wqkv_d[dt2 * P : (dt2 + 1) * P, D : D + 2 * P],
            )
        load_biases()


        qkT = {}

        def qkv_pair(hp):
            """qkT tiles for pair hp: q (scaled) and k, 2 rotating slots."""
            qt = qk_pool.tile([P, S], BF, name=f"q{hp}", tag="qk")
            kt = qk_pool.tile([P, S], BF, name=f"k{hp}", tag="qk")
            qkT[hp] = (qt, kt)
            for nt, dst in ((hp, qt), (8 + hp, kt)):
                ps = psum.tile([P, S], F32, name="ps_qk", tag="big")
                for sh in range(2):
                    for dt2 in range(DT):
                        if hp < 2:
                            w_ap = WqkE[dt2][
                                :, ((nt >= 8) * 2 + hp) * P : ((nt >= 8) * 2 + hp + 1) * P
                            ]
                        else:
                            col = (nt - 2) if nt < 8 else (nt - 4)
                            w_ap = Wqk[dt2][:, col * P : (col + 1) * P]
                        nc.tensor.matmul(
                            ps[:, sh * 512 : (sh + 1) * 512],
                            w_ap,
                            xT[dt2][:, sh * 512 : (sh + 1) * 512],
                            start=(dt2 == 0),
                            stop=(dt2 == DT - 1),
                        )
                if nt < 8:  # q: (psum + b) * scale
                    nc.vector.tensor_scalar(
                        dst[:], ps[:], bqcol[:, nt : nt + 1], SCALE,
                        AluOp.add, AluOp.mult,
                    )
                else:  # k: psum + b
                    nc.vector.tensor_scalar_add(dst[:], ps[:], bqcol[:, nt : nt + 1])

        def v_phase(Wv):
            # v = x @ Wv + bv, head-interleaved into va
            for s8 in range(ST):
                ps = psum.tile([P, S], F32, name="ps_v", tag="big")
                for sh in range(2):
                    for dt2 in range(DT):
                        nc.tensor.matmul(
                            ps[:, sh * 512 : (sh + 1) * 512],
                            xT[dt2][:, s8 * P : (s8 + 1) * P],
                            Wv[dt2][:, sh * 512 : (sh + 1) * 512],
                            start=(dt2 == 0),
                            stop=(dt2 == DT - 1),
                        )
                nc.vector.tensor_copy(
                    va[s8][:].rearrange("p (h c) -> p h c", c=HD + 1)[:, :, 0:HD],
                    ps[:].rearrange("p (h c) -> p h c", c=HD),
                )

        exp_tiles = {}

        def scores_pair(hp, exp_pool):
            """scoresT + exp for heads (2hp, 2hp+1); fills exp_tiles[hp]."""
            expA = exp_pool.tile([P, ST * S], BF, name="expA", tag="expA")
            expB = exp_pool.tile([P, ST * S], BF, name="expB", tag="expB")
            exp_tiles[hp] = (expA, expB)
            qtile, ktile = qkT[hp]
            for jt in range(ST):
                psA = psum.tile([P, S], F32, name="psA", tag="big")
                psB = psum.tile([P, S], F32, name="psB", tag="big")
                for sh in range(2):
                    nc.tensor.matmul(
                        psA[:, sh * 512 : (sh + 1) * 512],
                        ktile[0:64, jt * P : (jt + 1) * P],
                        qtile[0:64, sh * 512 : (sh + 1) * 512],
                        tile_position=(0, 0),
                    )
                    nc.tensor.matmul(
                        psB[:, sh * 512 : (sh + 1) * 512],
                        ktile[64:128, jt * P : (jt + 1) * P],
                        qtile[64:128, sh * 512 : (sh + 1) * 512],
                        tile_position=(64, 0),
                    )
                nc.scalar.activation(
                    expA[:, jt * S : (jt + 1) * S], psA[:], Act.Exp, bias=zbias[:]
                )
                nc.scalar.activation(
                    expB[:, jt * S : (jt + 1) * S], psB[:], Act.Exp, bias=zbias[:]
                )

        def attnv_pair(hp):
            expA, expB = exp_tiles.pop(hp)
            for qh in range(2):
                for (ex, head) in ((expA, 2 * hp), (expB, 2 * hp + 1)):
                    pso = psmall.tile([HD + 1, 512], F32, name="pso", tag="pso")
                    for jt in range(ST):
                        nc.tensor.matmul(
                            pso[:],
                            va[jt][:, head * 65 : head * 65 + 65],
                            ex[:, jt * S + qh * 512 : jt * S + qh * 512 + 512],
                            start=(jt == 0),
                            stop=(jt == ST - 1),
                        )
                    po = (head % 2) * 64
                    reg = outT[hp][po : po + 64, qh * 512 : (qh + 1) * 512]
                    # copy unnormalized rows + Z out fast to release PSUM
                    nc.vector.tensor_copy(reg, pso[0:64, :])
                    zs = small.tile([1, 512], F32, name="zs", tag="zs")
                    nc.vector.tensor_copy(zs[:], pso[64:65, :])
                    rz = small.tile([1, 512], F32, name="rz", tag="rz")
                    nc.vector.reciprocal_approx_fast(out=rz[:], in_=zs[:])
                    bz = small.tile([P, 512], F32, name="bz", tag="bz")
                    nc.gpsimd.partition_broadcast(bz[:], rz[:])
                    nc.vector.tensor_mul(reg, reg, bz[po : po + 64, :])

        # ---- output projection (defs; emitted at schedule time) ----
        ob_pool = ctx.enter_context(tc.tile_pool(name="obp", bufs=2))
        # Wp reuses the Wqk slots (dead after qkv pair 7) via shared tags
        Wp = [
            persist.tile([P, D], BF, name=f"Wp{t}", tag=f"wsh{t}") for t in range(DT)
        ]

        def load_wp():
            for dt2 in range(DT):
                nc.sync.dma_start(Wp[dt2][:], wproj_d[dt2 * P : (dt2 + 1) * P, :])

        proj_ps = {}

        def proj_partial():
            # pairs 0-6 are done; accumulate their proj contribution for
            # st 0,1 while the PE would otherwise wait on exp(7)
            for st in range(2):
                ps = psum.tile([P, S], F32, name="ps_p", tag="big")
                proj_ps[st] = ps
                for sh in range(2):
                    for kt in range(DT - 1):
                        nc.tensor.matmul(
                            ps[:, sh * 512 : (sh + 1) * 512],
                            outT[kt][:, st * P : (st + 1) * P],
                            Wp[kt][:, sh * 512 : (sh + 1) * 512],
                            start=(kt == 0),
                            stop=False,
                        )

        def proj_emit(st, ps, kt0):
            for sh in range(2):
                for kt in range(kt0, DT):
                    nc.tensor.matmul(
                        ps[:, sh * 512 : (sh + 1) * 512],
                        outT[kt][:, st * P : (st + 1) * P],
                        Wp[kt][:, sh * 512 : (sh + 1) * 512],
                        start=(kt == 0),
                        stop=(kt == DT - 1),
                    )
            ob = ob_pool.tile([P, S], F32, name="ob", tag="ob")
            for sh in range(2):
                nc.vector.tensor_copy(
                    ob[:, sh * 512 : (sh + 1) * 512], ps[:, sh * 512 : (sh + 1) * 512]
                )
                nc.sync.dma_start(
                    out_d[st * P : (st + 1) * P, sh * 512 : (sh + 1) * 512],
                    ob[:, sh * 512 : (sh + 1) * 512],
                )

        # ---- software-pipelined schedule ----
        with tc.tile_pool(name="xv", bufs=1) as xv_pool, \
             tc.tile_pool(name="exp", bufs=2) as exp_pool:
            Wv = [xv_pool.tile([P, D], BF, name=f"Wv{t}") for t in range(DT)]
            for dt2 in range(DT):
                nc.sync.dma_start(
                    Wv[dt2][:], wqkv_d[dt2 * P : (dt2 + 1) * P, 2 * D :]
                )
            # bulk Wqk (pairs 2-7) after Wv: needed only ~70us in
            for dt2 in range(DT):
                nc.sync.dma_start(
                    Wqk[dt2][:, 0 : D - 2 * P],
                    wqkv_d[dt2 * P : (dt2 + 1) * P, 2 * P : D],
                )
                nc.sync.dma_start(
                    Wqk[dt2][:, D - 2 * P :],
                    wqkv_d[dt2 * P : (dt2 + 1) * P, D + 2 * P : 2 * D],
                )
            qkv_pair(0)
            qkv_pair(1)
            scores_pair(0, exp_pool)
            v_phase(Wv)
            for hp in range(1, 8):
                if hp + 1 < 8:
                    qkv_pair(hp + 1)
                scores_pair(hp, exp_pool)
                attnv_pair(hp - 1)
            load_wp()
            proj_partial()
            attnv_pair(7)
            for st in range(ST):
                if st in proj_ps:
                    proj_emit(st, proj_ps[st], DT - 1)
                else:
                    ps = psum.tile([P, S], F32, name="ps_p", tag="big")
                    proj_emit(st, ps, 0)

    nc.compile()
    _CACHE["nc"] = nc
    return nc


def kernel(x, W_qkv, b_qkv, W_proj, b_proj, _trace=False):
    nc = _build()
    from concourse.bass_utils import run_bass_kernel_spmd

    bf = ml_dtypes.bfloat16
    wq = np.ascontiguousarray(W_qkv, dtype=np.float32).astype(bf)
    wp = np.ascontiguousarray(W_proj, dtype=np.float32).astype(bf)
    bq0 = np.asarray(b_qkv, dtype=np.float32)
    bq = np.concatenate(
        [np.ascontiguousarray(bq0[:2048].reshape(16, 128).T).ravel(), bq0[2048:]]
    ).astype(np.float32)
    bp = np.ascontiguousarray(b_proj, dtype=np.float32)
    in_maps = []
    for i in range(N_CORES):
        in_maps.append(
            {
                "x": np.ascontiguousarray(np.asarray(x[i], dtype=np.float32).T).astype(bf),
                "W_qkv": wq,
                "b_qkv": bq,
                "W_proj": wp,
                "b_proj": bp,
            }
        )
    res = run_bass_kernel_spmd(
        nc, in_maps, core_ids=list(range(N_CORES)), trace=_trace
    )
    out = np.stack([res.results[i]["out"] for i in range(N_CORES)], axis=0).astype(
        np.float32
    )
    # v-bias and proj-bias applied exactly on the host:
    # out = (attn + 1*bv) @ Wp + bp  ==  attn @ Wp  +  (bv @ Wp + bp)
    corr = np.asarray(b_qkv, np.float32)[2 * D :] @ np.asarray(W_proj, np.float32)
    corr = corr + np.asarray(b_proj, np.float32)
    if np.any(corr):
        out += corr[None, None, :]
    if _trace:
        _CACHE["last_results"] = res
    return out
